# revision 20
# baseline (speedup 1.0000x reference)
"""Causal self-attention (RoPE) Trainium2 kernel.

Sharding: 2 batches x 16 heads = 32 (b,h) units over 8 cores -> each core
handles 1 batch x 4 heads. Column-parallel QKV + row-parallel output
projection; host sums the 4 partial outputs per batch.

All matmul operands are fp16 (1 cycle/row on the PE); fp32 PSUM accumulation.

Design notes (v2, PE-continuity focused):
  - Attention runs on 128-wide q chunks (qtc 0..15), nkc = qtc+1 key chunks
    of 128 -> only 53% of the full score matrix is computed (vs 62.5% with
    512-wide q tiles).
  - The causal mask for the single diagonal [128,128] block is applied ON
    THE PE as an extra accumulating matmul (iden^T @ masktri) into the S
    PSUM, so the S -> exp chain never touches the Vector engine.
  - exp is batched: up to 4 S subtiles share one [128,512] PSUM bank and
    one scalar-engine activation.
  - RoPE: two PSUM-reading multiplies + the pair-swap shuffle on Vector;
    the final add runs on GpSimd (idle otherwise).
  - Softmax denominator: ones-column matmuls accumulate [1,128] per q chunk;
    reciprocal on Vector, partition_broadcast on GpSimd, normalize multiply
    on Vector.
  - Emission interleaves att(h) with qk(h+1) and att(3) with the output
    projection so the PE always has dense matmul work queued behind
    exp-gated work (PE p-state drops to 1.2 GHz for 3 us after any stall).
"""

import sys

if "/opt/trn_rl_repo" not in sys.path:
    sys.path.insert(0, "/opt/trn_rl_repo")

import numpy as np

import concourse.bass as bass
import concourse.tile as tile
from concourse import bacc, mybir
from concourse.bass_utils import run_bass_kernel_spmd

F32 = mybir.dt.float32
F16 = mybir.dt.float16

B, T, C = 2, 2048, 2048
NH, HD = 16, 128
NHL = 4            # heads per core
D_LOC = NHL * HD   # 512 local head dims
N_CORES = 8
SCALE = 1.0 / float(np.sqrt(HD))
NEG = -30000.0     # big enough: exp((S+NEG)*SCALE) == 0 for |S| < ~1000

CC = C // 128      # 16 contraction chunks
KC = T // 128      # 16 key chunks
NQC = T // 128     # 16 q chunks (128 wide)

_compiled = None


def _build():
    nc = bacc.Bacc("TRN2", target_bir_lowering=False, debug=False)

    xT_d = nc.dram_tensor("xT", [C, T], F16, kind="ExternalInput")
    wq_d = nc.dram_tensor("wq", [NHL, 128, CC, 128], F16, kind="ExternalInput")
    wk_d = nc.dram_tensor("wk", [NHL, 128, CC, 128], F16, kind="ExternalInput")
    wv_d = nc.dram_tensor("wv", [128, CC, D_LOC], F16, kind="ExternalInput")
    w2_d = nc.dram_tensor("w2", [128, NHL, C], F16, kind="ExternalInput")
    cos2_d = nc.dram_tensor("cos2", [128, T], F16, kind="ExternalInput")
    sin2s_d = nc.dram_tensor("sin2s", [128, T], F16, kind="ExternalInput")
    masktri_d = nc.dram_tensor("masktri", [128, 128], F16, kind="ExternalInput")
    iden_d = nc.dram_tensor("iden", [128, 128], F16, kind="ExternalInput")
    out_d = nc.dram_tensor("out", [T, C], F16, kind="ExternalOutput")

    swap_mask = list(range(16, 32)) + list(range(16))

    with tile.TileContext(nc) as tc, \
         tc.tile_pool(name="persist", bufs=1) as persist, \
         tc.tile_pool(name="pw", bufs=4) as pw, \
         tc.tile_pool(name="rope", bufs=2) as prope, \
         tc.tile_pool(name="att", bufs=6) as patt, \
         tc.tile_pool(name="nrm", bufs=4) as pnrm, \
         tc.tile_pool(name="outp", bufs=2) as pout, \
         tc.tile_pool(name="pbig", bufs=5, space="PSUM") as pbig, \
         tc.tile_pool(name="psy", bufs=2, space="PSUM") as psy, \
         tc.tile_pool(name="psd", bufs=1, space="PSUM") as psd:
        # persistent tiles
        xs = persist.tile([128, CC, T], F16, tag="xs")
        # qkT split per (jc, tt) window and yT per (h, qc) chunk so every
        # producer/consumer dependency is exact (no reliance on subtile
        # tracking when emission interleaves phases).
        qkTt = {}
        for jc in range(8):
            for tt in range(4):
                qkTt[(jc, tt)] = persist.tile(
                    [128, 512], F16, tag=f"qkT{jc}_{tt}",
                    name=f"qkT{jc}_{tt}")
        yTt = {}
        for h in range(NHL):
            for qc in range(NQC):
                yTt[(h, qc)] = persist.tile(
                    [128, 128], F16, tag=f"yT{h}_{qc}", name=f"yT{h}_{qc}")
        v_sb = persist.tile([128, KC, D_LOC], F16, tag="vsb")

        def qk_ap(jc, t0, t1):
            # [t0, t1) must lie within one 512 window
            tt = t0 // 512
            return qkTt[(jc, tt)][:, t0 - tt * 512:t1 - tt * 512]
        masktri_sb = persist.tile([128, 128], F16, tag="masktri")
        iden_sb = persist.tile([128, 128], F16, tag="iden")
        cos2 = persist.tile([128, T], F16, tag="cos2")
        sin2s = persist.tile([128, T], F16, tag="sin2s")
        w2_sb = persist.tile([128, NHL, C], F16, tag="w2")
        wv_sb = persist.tile([128, CC, D_LOC], F16, tag="wv")
        allones_sb = persist.tile([128, 128], F16, tag="allones")
        nc.vector.memset(allones_sb, 1.0)

        def emit_xs_dma(tq, eng=None):
            eng = eng or nc.sync
            t0, t1 = tq * 512, (tq + 1) * 512
            for cc in range(CC):
                eng.dma_start(
                    out=xs[:, cc, t0:t1],
                    in_=xT_d.ap()[cc * 128:(cc + 1) * 128, t0:t1],
                )

        def emit_trig_dma(eng=None):
            eng = eng or nc.sync
            for half in range(2):
                t0, t1 = half * (T // 2), (half + 1) * (T // 2)
                eng.dma_start(out=cos2[:, t0:t1],
                              in_=cos2_d.ap()[:, t0:t1])
                eng.dma_start(out=sin2s[:, t0:t1],
                              in_=sin2s_d.ap()[:, t0:t1])

        def emit_wv_dma(eng=None):
            eng = eng or nc.sync
            for g in range(8):  # wv in 8 chunks of 2 cc
                eng.dma_start(out=wv_sb[:, 2 * g:2 * (g + 1), :],
                              in_=wv_d.ap()[:, 2 * g:2 * (g + 1), :])

        def emit_misc_dma():
            nc.sync.dma_start(out=masktri_sb, in_=masktri_d.ap())
            nc.sync.dma_start(out=iden_sb, in_=iden_d.ap())
            for hh in range(NHL):  # w2 in 4 chunks
                nc.sync.dma_start(out=w2_sb[:, hh, :],
                                  in_=w2_d.ap()[:, hh, :])

        # ---------------- V = x @ Wv, [t, d] layout ----------------
        def emit_v():
            for tch in range(KC):
                pv = pbig.tile([128, 512], F32, tag="big", name=f"pv{tch}")
                for cc in range(CC):
                    nc.tensor.matmul(
                        pv,
                        xs[:, cc, tch * 128:(tch + 1) * 128],
                        wv_sb[:, cc, :],
                        start=(cc == 0), stop=(cc == CC - 1),
                    )
                nc.scalar.copy(v_sb[:, tch, :], pv)

        # ---------------- QK^T tiles + RoPE ----------------
        w_tiles = {}

        def emit_qk_dma(h):
            for jc in (h, 4 + h):
                w_src = (wq_d if jc < 4 else wk_d).ap()[jc % 4]
                w_sb = pw.tile([128, CC, 128], F16, tag="w",
                               name=f"w_sb{jc}")
                for g in range(4):
                    nc.sync.dma_start(out=w_sb[:, 4 * g:4 * (g + 1), :],
                                      in_=w_src[:, 4 * g:4 * (g + 1), :])
                w_tiles[jc] = w_sb

        def qk_tile(jc, tt):
            w_sb = w_tiles[jc]
            gt0 = tt * 512
            ps = pbig.tile([128, 512], F32, tag="big", name=f"psqk{jc}_{tt}")
            for cc in range(CC):
                nc.tensor.matmul(
                    ps, w_sb[:, cc, :],
                    xs[:, cc, gt0:gt0 + 512],
                    start=(cc == 0), stop=(cc == CC - 1),
                )
            u = prope.tile([128, 512], F16, tag="u", name=f"u{jc}{tt}")
            v = prope.tile([128, 512], F16, tag="v", name=f"v{jc}{tt}")
            w = prope.tile([128, 512], F16, tag="w", name=f"w{jc}{tt}")
            nc.vector.tensor_mul(u, ps, cos2[:, gt0:gt0 + 512])
            nc.vector.tensor_mul(v, ps, sin2s[:, gt0:gt0 + 512])
            nc.vector.stream_shuffle(w, v, swap_mask)
            nc.vector.tensor_add(qkTt[(jc, tt)], u, w)

        def qk_emitters(h):
            return [(lambda jc=jc, tt=tt: qk_tile(jc, tt))
                    for jc in (h, 4 + h) for tt in range(4)]

        # ---------------- attention, 128-wide q chunks ----------------
        def att_units(h):
            """Returns list of (emit_S, emit_YD) closure pairs."""
            units = []
            state = {}

            def make_S(qtc, gi, group):
                def emit_S():
                    sps4 = pbig.tile([128, 512], F32, tag="big",
                                     name=f"sps{h}_{qtc}_{gi}")
                    a4 = patt.tile([128, 512], F16, tag="a",
                                   name=f"a{h}_{qtc}_{gi}")
                    for j, kc in enumerate(group):
                        nc.tensor.matmul(
                            sps4[:, j * 128:(j + 1) * 128],
                            qk_ap(4 + h, kc * 128, (kc + 1) * 128),
                            qk_ap(h, qtc * 128, (qtc + 1) * 128),
                            start=True, stop=(kc != qtc),
                        )
                        if kc == qtc:
                            nc.tensor.matmul(
                                sps4[:, j * 128:(j + 1) * 128],
                                iden_sb, masktri_sb,
                                start=False, stop=True,
                            )
                    wdt = len(group) * 128
                    nc.scalar.activation(
                        a4[:, :wdt], sps4[:, :wdt],
                        mybir.ActivationFunctionType.Exp, scale=SCALE,
                    )
                    state[(qtc, gi)] = a4
                return emit_S

            def make_YD(qtc, gi, group, ngroups):
                def emit_YD():
                    if gi == 0:
                        state[("y", qtc)] = psy.tile(
                            [128, 128], F32, tag="y", name=f"yps{h}_{qtc}")
                        state[("d", qtc)] = psd.tile(
                            [128, 128], F32, tag="d", name=f"dps{h}_{qtc}")
                    yps = state[("y", qtc)]
                    dps = state[("d", qtc)]
                    a4 = state.pop((qtc, gi))
                    for j, kc in enumerate(group):
                        nc.tensor.matmul(
                            yps, v_sb[:, kc, h * HD:(h + 1) * HD],
                            a4[:, j * 128:(j + 1) * 128],
                            start=(kc == 0), stop=(kc == qtc),
                        )
                    # all-ones lhsT: every output row = the column sums, so
                    # dps is the denominator already broadcast to 128 rows
                    for j, kc in enumerate(group):
                        nc.tensor.matmul(
                            dps, allones_sb,
                            a4[:, j * 128:(j + 1) * 128],
                            start=(kc == 0), stop=(kc == qtc),
                        )
                    if gi == ngroups - 1:
                        rb = pnrm.tile([128, 128], F32, tag="rb",
                                       name=f"rb{h}{qtc}")
                        nc.vector.reciprocal_approx_fast(rb, dps)
                        nc.vector.tensor_mul(yTt[(h, qtc)], yps, rb)
                return emit_YD

            for qtc in range(NQC):
                kcs = list(range(qtc + 1))
                groups = [kcs[i:i + 4] for i in range(0, len(kcs), 4)]
                for gi, group in enumerate(groups):
                    units.append((make_S(qtc, gi, group),
                                  make_YD(qtc, gi, group, len(groups))))
            return units

        # ---------------- output projection ----------------
        def proj_unit(qc):
            for ct in range(C // 512):
                ops = pbig.tile([128, 512], F32, tag="big",
                                name=f"ops{qc}{ct}")
                for hh in range(NHL):
                    nc.tensor.matmul(
                        ops,
                        yTt[(hh, qc)],
                        w2_sb[:, hh, ct * 512:(ct + 1) * 512],
                        start=(hh == 0), stop=(hh == NHL - 1),
                    )
                osb = pout.tile([128, 512], F16, tag="o",
                                name=f"osb{qc}{ct}")
                # drain half on Vector, half on Scalar: halves the latency
                # until the PSUM buf frees and splits the copy load
                nc.vector.tensor_copy(osb[:, 0:256], ops[:, 0:256])
                nc.scalar.copy(osb[:, 256:512], ops[:, 256:512])
                dst = out_d.ap()[qc * 128:(qc + 1) * 128,
                                 ct * 512:(ct + 1) * 512]
                # out DMA triggers ride the idle GpSimd sequencer (SWDGE)
                if qc >= NQC - 2:
                    # split the last transfers across queues: shorter tail
                    for qq in range(2):
                        nc.gpsimd.dma_start(
                            out=dst[:, qq * 256:(qq + 1) * 256],
                            in_=osb[:, qq * 256:(qq + 1) * 256])
                else:
                    nc.gpsimd.dma_start(out=dst, in_=osb)

        # ---------------- software-pipelined emission ----------------
        LOOK = 3

        def run_phase(units, extras, period):
            """Pipeline att units (S ahead of YD by LOOK); sprinkle dense
            `extras` closures every `period` units to cover exp latency."""
            n = len(units)
            ei = 0
            step = 0
            for i in range(n + LOOK):
                if i < n:
                    units[i][0]()
                if i >= LOOK:
                    units[i - LOOK][1]()
                step += 1
                if extras and step % period == 0 and ei < len(extras):
                    extras[ei]()
                    ei += 1
            while extras and ei < len(extras):
                extras[ei]()
                ei += 1

        def run_tail(units, ends, look=2):
            # shallower lookahead: leaves a PSUM buf free so proj tiles
            # double-buffer against their drain latency
            n = len(units)
            done_proj = 0
            for i in range(n + look):
                if i < n:
                    units[i][0]()
                if i >= look:
                    units[i - look][1]()
                while done_proj < NQC and i - look + 1 >= ends[done_proj]:
                    proj_unit(done_proj)
                    done_proj += 1
            while done_proj < NQC:
                proj_unit(done_proj)
                done_proj += 1

        # --- prologue: spread DMA triggers over idle sequencers (565-667ns
        # per trigger serializes; SP alone would gate the start) ---
        for jc in (0, 4):
            w_tiles[jc] = pw.tile([128, CC, 128], F16, tag="w",
                                  name=f"w_sb{jc}")
        for g in range(4):  # SP: head-0 weights + xs q0, first-needed first
            for jc in (0, 4):
                w_src = (wq_d if jc < 4 else wk_d).ap()[jc % 4]
                nc.sync.dma_start(out=w_tiles[jc][:, 4 * g:4 * (g + 1), :],
                                  in_=w_src[:, 4 * g:4 * (g + 1), :])
            for cc in range(4 * g, 4 * g + 4):
                nc.sync.dma_start(
                    out=xs[:, cc, 0:512],
                    in_=xT_d.ap()[cc * 128:(cc + 1) * 128, 0:512])
        emit_trig_dma(nc.sync)
        emit_wv_dma(nc.sync)
        emit_xs_dma(1, nc.scalar)
        emit_xs_dma(2, nc.scalar)
        emit_xs_dma(3, nc.gpsimd)
        emit_misc_dma()
        for tt in range(4):
            qk_tile(0, tt)
            qk_tile(4, tt)
        emit_v()
        for h in range(NHL - 1):
            emit_qk_dma(h + 1)
            run_phase(att_units(h), qk_emitters(h + 1), period=5)
        u3 = att_units(NHL - 1)
        ends = []
        acc = 0
        for q in range(NQC):
            acc += (q // 4) + 1
            ends.append(acc)
        run_tail(u3, ends)

    nc.compile()
    return nc


def _prep_core_inputs(core, x16, W_attn, W_proj, cos2, sin2s, masktri, iden):
    b = core // 4
    g = core % 4
    heads = [g * NHL + i for i in range(NHL)]
    # stream_shuffle permutes within 32-partition blocks only: lay out each
    # block as [re pairs 16b..16b+15 | im pairs 16b..16b+15]
    perm = np.concatenate(
        [np.r_[2 * (16 * blk + np.arange(16)),
               2 * (16 * blk + np.arange(16)) + 1]
         for blk in range(4)]
    )

    xT = np.ascontiguousarray(x16[b].T)

    def qk_blocks(base):
        blocks = []
        for h in heads:
            blk = W_attn[:, base + h * HD: base + (h + 1) * HD][:, perm]
            blocks.append(blk.reshape(CC, 128, HD).transpose(1, 0, 2))
        return np.ascontiguousarray(np.stack(blocks, axis=0)).astype(np.float16)

    wq = qk_blocks(0)
    wk = qk_blocks(C)
    wv = np.concatenate(
        [W_attn[:, 2 * C + h * HD: 2 * C + (h + 1) * HD] for h in heads],
        axis=1,
    )  # (C, D_LOC)
    wv = np.ascontiguousarray(
        wv.reshape(CC, 128, D_LOC).transpose(1, 0, 2)).astype(np.float16)
    w2 = np.ascontiguousarray(
        np.stack([W_proj[h * HD:(h + 1) * HD, :] for h in heads], axis=0)
        .transpose(1, 0, 2)
    ).astype(np.float16)
    return {
        "xT": xT, "wq": wq, "wk": wk, "wv": wv, "w2": w2,
        "cos2": cos2, "sin2s": sin2s, "masktri": masktri, "iden": iden,
    }


def _run(inputs, trace=False):
    global _compiled
    x = np.asarray(inputs["x"], dtype=np.float32)
    W_attn = np.asarray(inputs["W_attn"], dtype=np.float32)
    W_proj = np.asarray(inputs["W_proj"], dtype=np.float32)
    fc = np.asarray(inputs["freqs_cos"], dtype=np.float32)
    fs = np.asarray(inputs["freqs_sin"], dtype=np.float32)

    x16 = x.astype(np.float16)

    cosT = np.ascontiguousarray(fc.T)            # (64, T)
    sinT = np.ascontiguousarray(fs.T)
    # per 32-partition block b: partitions [0:16] carry cos/sin of pairs
    # 16b..16b+15 (re half, +sin), [16:32] the same freqs (im half, -sin)
    cos2 = np.concatenate(
        [np.concatenate([cosT[16 * blk:16 * (blk + 1)]] * 2, axis=0)
         for blk in range(4)], axis=0)           # (128, T)
    sin2s = np.concatenate(
        [np.concatenate([sinT[16 * blk:16 * (blk + 1)],
                         -sinT[16 * blk:16 * (blk + 1)]], axis=0)
         for blk in range(4)], axis=0)
    cos2 = np.ascontiguousarray(cos2).astype(np.float16)
    sin2s = np.ascontiguousarray(sin2s).astype(np.float16)

    ki = np.arange(128)[:, None]
    qi = np.arange(128)[None, :]
    masktri = np.ascontiguousarray(
        np.where(ki <= qi, 0.0, NEG).astype(np.float16))  # (128, 128)
    iden = np.ascontiguousarray(np.eye(128, dtype=np.float16))

    if _compiled is None:
        _compiled = _build()
    nc = _compiled

    in_maps = [
        _prep_core_inputs(c, x16, W_attn, W_proj, cos2, sin2s, masktri, iden)
        for c in range(N_CORES)
    ]
    res = run_bass_kernel_spmd(
        nc, in_maps, core_ids=list(range(N_CORES)), trace=trace)

    out = np.zeros((B, T, C), dtype=np.float32)
    for c in range(N_CORES):
        out[c // 4] += res.results[c]["out"]
    return out, res


def kernel(**inputs) -> np.ndarray:
    out, _ = _run(inputs, trace=False)
    return out


# revision 23
# speedup vs baseline: 1.1243x; 1.1243x over previous
"""Causal self-attention (RoPE) Trainium2 kernel.

Sharding: 2 batches x 16 heads = 32 (b,h) units over 8 cores -> each core
handles 1 batch x 4 heads. Column-parallel QKV + row-parallel output
projection; host sums the 4 partial outputs per batch.

All matmul operands are fp16 (1 cycle/row on the PE); fp32 PSUM accumulation.

Design notes (v2, PE-continuity focused):
  - Attention runs on 128-wide q chunks (qtc 0..15), nkc = qtc+1 key chunks
    of 128 -> only 53% of the full score matrix is computed (vs 62.5% with
    512-wide q tiles).
  - The causal mask for the single diagonal [128,128] block is applied ON
    THE PE as an extra accumulating matmul (iden^T @ masktri) into the S
    PSUM, so the S -> exp chain never touches the Vector engine.
  - exp is batched: up to 4 S subtiles share one [128,512] PSUM bank and
    one scalar-engine activation.
  - RoPE: two PSUM-reading multiplies + the pair-swap shuffle on Vector;
    the final add runs on GpSimd (idle otherwise).
  - Softmax denominator: ones-column matmuls accumulate [1,128] per q chunk;
    reciprocal on Vector, partition_broadcast on GpSimd, normalize multiply
    on Vector.
  - Emission interleaves att(h) with qk(h+1) and att(3) with the output
    projection so the PE always has dense matmul work queued behind
    exp-gated work (PE p-state drops to 1.2 GHz for 3 us after any stall).
"""

import sys

if "/opt/trn_rl_repo" not in sys.path:
    sys.path.insert(0, "/opt/trn_rl_repo")

import numpy as np

import concourse.bass as bass
import concourse.tile as tile
from concourse import bacc, mybir
from concourse.bass_utils import run_bass_kernel_spmd

F32 = mybir.dt.float32
F16 = mybir.dt.float16

B, T, C = 2, 2048, 2048
NH, HD = 16, 128
NHL = 4            # heads per core
D_LOC = NHL * HD   # 512 local head dims
N_CORES = 8
SCALE = 1.0 / float(np.sqrt(HD))
NEG = -30000.0     # big enough: exp((S+NEG)*SCALE) == 0 for |S| < ~1000

CC = C // 128      # 16 contraction chunks
KC = T // 128      # 16 key chunks
NQC = T // 128     # 16 q chunks (128 wide)

_compiled = None


def _build():
    nc = bacc.Bacc("TRN2", target_bir_lowering=False, debug=False)

    xT_d = nc.dram_tensor("xT", [C, T], F16, kind="ExternalInput")
    wq_d = nc.dram_tensor("wq", [NHL, 128, CC, 128], F16, kind="ExternalInput")
    wk_d = nc.dram_tensor("wk", [NHL, 128, CC, 128], F16, kind="ExternalInput")
    wv_d = nc.dram_tensor("wv", [128, CC, D_LOC], F16, kind="ExternalInput")
    w2_d = nc.dram_tensor("w2", [128, NHL, C], F16, kind="ExternalInput")
    cos2_d = nc.dram_tensor("cos2", [128, T], F16, kind="ExternalInput")
    sin2s_d = nc.dram_tensor("sin2s", [128, T], F16, kind="ExternalInput")
    masktri_d = nc.dram_tensor("masktri", [128, 128], F16, kind="ExternalInput")
    iden_d = nc.dram_tensor("iden", [128, 128], F16, kind="ExternalInput")
    out_d = nc.dram_tensor("out", [T, C], F16, kind="ExternalOutput")

    swap_mask = list(range(16, 32)) + list(range(16))

    with tile.TileContext(nc) as tc, \
         tc.tile_pool(name="persist", bufs=1) as persist, \
         tc.tile_pool(name="pw", bufs=4) as pw, \
         tc.tile_pool(name="rope", bufs=2) as prope, \
         tc.tile_pool(name="att", bufs=6) as patt, \
         tc.tile_pool(name="nrm", bufs=4) as pnrm, \
         tc.tile_pool(name="outp", bufs=4) as pout, \
         tc.tile_pool(name="pbig", bufs=5, space="PSUM") as pbig, \
         tc.tile_pool(name="psy", bufs=2, space="PSUM") as psy, \
         tc.tile_pool(name="psd", bufs=1, space="PSUM") as psd:
        # persistent tiles
        xs = persist.tile([128, CC, T], F16, tag="xs")
        # qkT split per (jc, tt) window and yT per (h, qc) chunk so every
        # producer/consumer dependency is exact (no reliance on subtile
        # tracking when emission interleaves phases).
        qkTt = {}
        for jc in range(8):
            for tt in range(4):
                qkTt[(jc, tt)] = persist.tile(
                    [128, 512], F16, tag=f"qkT{jc}_{tt}",
                    name=f"qkT{jc}_{tt}")
        yTt = {}
        for h in range(NHL):
            for qc in range(NQC):
                yTt[(h, qc)] = persist.tile(
                    [128, 128], F16, tag=f"yT{h}_{qc}", name=f"yT{h}_{qc}")
        v_sb = persist.tile([128, KC, D_LOC], F16, tag="vsb")

        def qk_ap(jc, t0, t1):
            # [t0, t1) must lie within one 512 window
            tt = t0 // 512
            return qkTt[(jc, tt)][:, t0 - tt * 512:t1 - tt * 512]
        masktri_sb = persist.tile([128, 128], F16, tag="masktri")
        iden_sb = persist.tile([128, 128], F16, tag="iden")
        cos2 = persist.tile([128, T], F16, tag="cos2")
        sin2s = persist.tile([128, T], F16, tag="sin2s")
        w2_sb = persist.tile([128, NHL, C], F16, tag="w2")
        wv_sb = persist.tile([128, CC, D_LOC], F16, tag="wv")
        allones_sb = persist.tile([128, 128], F16, tag="allones")
        nc.vector.memset(allones_sb, 1.0)

        def emit_xs_dma(tq, eng=None):
            eng = eng or nc.sync
            t0, t1 = tq * 512, (tq + 1) * 512
            for cc in range(CC):
                eng.dma_start(
                    out=xs[:, cc, t0:t1],
                    in_=xT_d.ap()[cc * 128:(cc + 1) * 128, t0:t1],
                )

        def emit_trig_dma(eng=None):
            eng = eng or nc.sync
            for half in range(2):
                t0, t1 = half * (T // 2), (half + 1) * (T // 2)
                eng.dma_start(out=cos2[:, t0:t1],
                              in_=cos2_d.ap()[:, t0:t1])
                eng.dma_start(out=sin2s[:, t0:t1],
                              in_=sin2s_d.ap()[:, t0:t1])

        def emit_wv_dma(eng=None):
            eng = eng or nc.sync
            for g in range(8):  # wv in 8 chunks of 2 cc
                eng.dma_start(out=wv_sb[:, 2 * g:2 * (g + 1), :],
                              in_=wv_d.ap()[:, 2 * g:2 * (g + 1), :])

        def emit_misc_dma():
            nc.sync.dma_start(out=masktri_sb, in_=masktri_d.ap())
            nc.sync.dma_start(out=iden_sb, in_=iden_d.ap())
            for hh in range(NHL):  # w2 in 4 chunks
                nc.sync.dma_start(out=w2_sb[:, hh, :],
                                  in_=w2_d.ap()[:, hh, :])

        # ---------------- V = x @ Wv, [t, d] layout ----------------
        def emit_v():
            for tch in range(KC):
                pv = pbig.tile([128, 512], F32, tag="big", name=f"pv{tch}")
                for cc in range(CC):
                    nc.tensor.matmul(
                        pv,
                        xs[:, cc, tch * 128:(tch + 1) * 128],
                        wv_sb[:, cc, :],
                        start=(cc == 0), stop=(cc == CC - 1),
                    )
                nc.scalar.copy(v_sb[:, tch, :], pv)

        # ---------------- QK^T tiles + RoPE ----------------
        w_tiles = {}

        def emit_qk_dma(h):
            for jc in (h, 4 + h):
                w_src = (wq_d if jc < 4 else wk_d).ap()[jc % 4]
                w_sb = pw.tile([128, CC, 128], F16, tag="w",
                               name=f"w_sb{jc}")
                for g in range(4):
                    nc.sync.dma_start(out=w_sb[:, 4 * g:4 * (g + 1), :],
                                      in_=w_src[:, 4 * g:4 * (g + 1), :])
                w_tiles[jc] = w_sb

        def qk_tile(jc, tt):
            w_sb = w_tiles[jc]
            gt0 = tt * 512
            ps = pbig.tile([128, 512], F32, tag="big", name=f"psqk{jc}_{tt}")
            for cc in range(CC):
                nc.tensor.matmul(
                    ps, w_sb[:, cc, :],
                    xs[:, cc, gt0:gt0 + 512],
                    start=(cc == 0), stop=(cc == CC - 1),
                )
            u = prope.tile([128, 512], F16, tag="u", name=f"u{jc}{tt}")
            v = prope.tile([128, 512], F16, tag="v", name=f"v{jc}{tt}")
            w = prope.tile([128, 512], F16, tag="w", name=f"w{jc}{tt}")
            nc.vector.tensor_mul(u, ps, cos2[:, gt0:gt0 + 512])
            nc.vector.tensor_mul(v, ps, sin2s[:, gt0:gt0 + 512])
            nc.vector.stream_shuffle(w, v, swap_mask)
            nc.vector.tensor_add(qkTt[(jc, tt)], u, w)

        def qk_emitters(h):
            return [(lambda jc=jc, tt=tt: qk_tile(jc, tt))
                    for jc in (h, 4 + h) for tt in range(4)]

        # ---------------- attention, 128-wide q chunks ----------------
        def att_units(h):
            """Returns list of (emit_S, emit_YD) closure pairs."""
            units = []
            state = {}

            def make_S(qtc, gi, group):
                def emit_S():
                    sps4 = pbig.tile([128, 512], F32, tag="big",
                                     name=f"sps{h}_{qtc}_{gi}")
                    a4 = patt.tile([128, 512], F16, tag="a",
                                   name=f"a{h}_{qtc}_{gi}")
                    for j, kc in enumerate(group):
                        nc.tensor.matmul(
                            sps4[:, j * 128:(j + 1) * 128],
                            qk_ap(4 + h, kc * 128, (kc + 1) * 128),
                            qk_ap(h, qtc * 128, (qtc + 1) * 128),
                            start=True, stop=(kc != qtc),
                        )
                        if kc == qtc:
                            nc.tensor.matmul(
                                sps4[:, j * 128:(j + 1) * 128],
                                iden_sb, masktri_sb,
                                start=False, stop=True,
                            )
                    wdt = len(group) * 128
                    nc.scalar.activation(
                        a4[:, :wdt], sps4[:, :wdt],
                        mybir.ActivationFunctionType.Exp, scale=SCALE,
                    )
                    state[(qtc, gi)] = a4
                return emit_S

            def make_YD(qtc, gi, group, ngroups):
                def emit_YD():
                    if gi == 0:
                        state[("y", qtc)] = psy.tile(
                            [128, 128], F32, tag="y", name=f"yps{h}_{qtc}")
                        state[("d", qtc)] = psd.tile(
                            [128, 128], F32, tag="d", name=f"dps{h}_{qtc}")
                    yps = state[("y", qtc)]
                    dps = state[("d", qtc)]
                    a4 = state.pop((qtc, gi))
                    for j, kc in enumerate(group):
                        nc.tensor.matmul(
                            yps, v_sb[:, kc, h * HD:(h + 1) * HD],
                            a4[:, j * 128:(j + 1) * 128],
                            start=(kc == 0), stop=(kc == qtc),
                        )
                    # all-ones lhsT: every output row = the column sums, so
                    # dps is the denominator already broadcast to 128 rows
                    for j, kc in enumerate(group):
                        nc.tensor.matmul(
                            dps, allones_sb,
                            a4[:, j * 128:(j + 1) * 128],
                            start=(kc == 0), stop=(kc == qtc),
                        )
                    if gi == ngroups - 1:
                        rb = pnrm.tile([128, 128], F32, tag="rb",
                                       name=f"rb{h}{qtc}")
                        nc.vector.reciprocal_approx_fast(rb, dps)
                        nc.vector.tensor_mul(yTt[(h, qtc)], yps, rb)
                return emit_YD

            for qtc in range(NQC):
                kcs = list(range(qtc + 1))
                groups = [kcs[i:i + 4] for i in range(0, len(kcs), 4)]
                for gi, group in enumerate(groups):
                    units.append((make_S(qtc, gi, group),
                                  make_YD(qtc, gi, group, len(groups))))
            return units

        # ---------------- output projection ----------------
        def proj_unit(qc):
            for ct in range(C // 512):
                ops = pbig.tile([128, 512], F32, tag="big",
                                name=f"ops{qc}{ct}")
                for hh in range(NHL):
                    nc.tensor.matmul(
                        ops,
                        yTt[(hh, qc)],
                        w2_sb[:, hh, ct * 512:(ct + 1) * 512],
                        start=(hh == 0), stop=(hh == NHL - 1),
                    )
                osb = pout.tile([128, 512], F16, tag="o",
                                name=f"osb{qc}{ct}")
                # drain half on Vector, half on Scalar: halves the latency
                # until the PSUM buf frees and splits the copy load
                nc.vector.tensor_copy(osb[:, 0:256], ops[:, 0:256])
                nc.scalar.copy(osb[:, 256:512], ops[:, 256:512])
                dst = out_d.ap()[qc * 128:(qc + 1) * 128,
                                 ct * 512:(ct + 1) * 512]
                if qc >= NQC - 2:
                    # split the last transfers across queues: shorter tail
                    for qq in range(2):
                        nc.sync.dma_start(
                            out=dst[:, qq * 256:(qq + 1) * 256],
                            in_=osb[:, qq * 256:(qq + 1) * 256])
                else:
                    nc.sync.dma_start(out=dst, in_=osb)

        # ---------------- software-pipelined emission ----------------
        LOOK = 3

        def run_phase(units, extras, period):
            """Pipeline att units (S ahead of YD by LOOK); sprinkle dense
            `extras` closures every `period` units to cover exp latency."""
            n = len(units)
            ei = 0
            step = 0
            for i in range(n + LOOK):
                if i < n:
                    units[i][0]()
                if i >= LOOK:
                    units[i - LOOK][1]()
                step += 1
                if extras and step % period == 0 and ei < len(extras):
                    extras[ei]()
                    ei += 1
            while extras and ei < len(extras):
                extras[ei]()
                ei += 1

        def run_tail(units, ends, look=2):
            # shallower lookahead: leaves a PSUM buf free so proj tiles
            # double-buffer against their drain latency
            n = len(units)
            done_proj = 0
            for i in range(n + look):
                if i < n:
                    units[i][0]()
                if i >= look:
                    units[i - look][1]()
                while done_proj < NQC and i - look + 1 >= ends[done_proj]:
                    proj_unit(done_proj)
                    done_proj += 1
            while done_proj < NQC:
                proj_unit(done_proj)
                done_proj += 1

        # --- prologue: spread DMA triggers over idle sequencers (565-667ns
        # per trigger serializes; SP alone would gate the start) ---
        for jc in (0, 4):
            w_tiles[jc] = pw.tile([128, CC, 128], F16, tag="w",
                                  name=f"w_sb{jc}")
        for g in range(4):  # SP: head-0 weights + xs q0, first-needed first
            for jc in (0, 4):
                w_src = (wq_d if jc < 4 else wk_d).ap()[jc % 4]
                nc.sync.dma_start(out=w_tiles[jc][:, 4 * g:4 * (g + 1), :],
                                  in_=w_src[:, 4 * g:4 * (g + 1), :])
            for cc in range(4 * g, 4 * g + 4):
                nc.sync.dma_start(
                    out=xs[:, cc, 0:512],
                    in_=xT_d.ap()[cc * 128:(cc + 1) * 128, 0:512])
        emit_trig_dma(nc.sync)
        emit_wv_dma(nc.sync)
        emit_xs_dma(1, nc.scalar)
        emit_xs_dma(2, nc.scalar)
        emit_xs_dma(3, nc.scalar)
        emit_misc_dma()
        for tt in range(4):
            qk_tile(0, tt)
            qk_tile(4, tt)
        emit_v()
        for h in range(NHL - 1):
            emit_qk_dma(h + 1)
            run_phase(att_units(h), qk_emitters(h + 1), period=5)
        u3 = att_units(NHL - 1)
        ends = []
        acc = 0
        for q in range(NQC):
            acc += (q // 4) + 1
            ends.append(acc)
        run_tail(u3, ends)

    nc.compile()
    return nc


def _prep_core_inputs(core, x16, W_attn, W_proj, cos2, sin2s, masktri, iden):
    b = core // 4
    g = core % 4
    heads = [g * NHL + i for i in range(NHL)]
    # stream_shuffle permutes within 32-partition blocks only: lay out each
    # block as [re pairs 16b..16b+15 | im pairs 16b..16b+15]
    perm = np.concatenate(
        [np.r_[2 * (16 * blk + np.arange(16)),
               2 * (16 * blk + np.arange(16)) + 1]
         for blk in range(4)]
    )

    xT = np.ascontiguousarray(x16[b].T)

    def qk_blocks(base):
        blocks = []
        for h in heads:
            blk = W_attn[:, base + h * HD: base + (h + 1) * HD][:, perm]
            blocks.append(blk.reshape(CC, 128, HD).transpose(1, 0, 2))
        return np.ascontiguousarray(np.stack(blocks, axis=0)).astype(np.float16)

    wq = qk_blocks(0)
    wk = qk_blocks(C)
    wv = np.concatenate(
        [W_attn[:, 2 * C + h * HD: 2 * C + (h + 1) * HD] for h in heads],
        axis=1,
    )  # (C, D_LOC)
    wv = np.ascontiguousarray(
        wv.reshape(CC, 128, D_LOC).transpose(1, 0, 2)).astype(np.float16)
    w2 = np.ascontiguousarray(
        np.stack([W_proj[h * HD:(h + 1) * HD, :] for h in heads], axis=0)
        .transpose(1, 0, 2)
    ).astype(np.float16)
    return {
        "xT": xT, "wq": wq, "wk": wk, "wv": wv, "w2": w2,
        "cos2": cos2, "sin2s": sin2s, "masktri": masktri, "iden": iden,
    }


def _run(inputs, trace=False):
    global _compiled
    x = np.asarray(inputs["x"], dtype=np.float32)
    W_attn = np.asarray(inputs["W_attn"], dtype=np.float32)
    W_proj = np.asarray(inputs["W_proj"], dtype=np.float32)
    fc = np.asarray(inputs["freqs_cos"], dtype=np.float32)
    fs = np.asarray(inputs["freqs_sin"], dtype=np.float32)

    x16 = x.astype(np.float16)

    cosT = np.ascontiguousarray(fc.T)            # (64, T)
    sinT = np.ascontiguousarray(fs.T)
    # per 32-partition block b: partitions [0:16] carry cos/sin of pairs
    # 16b..16b+15 (re half, +sin), [16:32] the same freqs (im half, -sin)
    cos2 = np.concatenate(
        [np.concatenate([cosT[16 * blk:16 * (blk + 1)]] * 2, axis=0)
         for blk in range(4)], axis=0)           # (128, T)
    sin2s = np.concatenate(
        [np.concatenate([sinT[16 * blk:16 * (blk + 1)],
                         -sinT[16 * blk:16 * (blk + 1)]], axis=0)
         for blk in range(4)], axis=0)
    cos2 = np.ascontiguousarray(cos2).astype(np.float16)
    sin2s = np.ascontiguousarray(sin2s).astype(np.float16)

    ki = np.arange(128)[:, None]
    qi = np.arange(128)[None, :]
    masktri = np.ascontiguousarray(
        np.where(ki <= qi, 0.0, NEG).astype(np.float16))  # (128, 128)
    iden = np.ascontiguousarray(np.eye(128, dtype=np.float16))

    if _compiled is None:
        _compiled = _build()
    nc = _compiled

    in_maps = [
        _prep_core_inputs(c, x16, W_attn, W_proj, cos2, sin2s, masktri, iden)
        for c in range(N_CORES)
    ]
    res = run_bass_kernel_spmd(
        nc, in_maps, core_ids=list(range(N_CORES)), trace=trace)

    out = np.zeros((B, T, C), dtype=np.float32)
    for c in range(N_CORES):
        out[c // 4] += res.results[c]["out"]
    return out, res


def kernel(**inputs) -> np.ndarray:
    out, _ = _run(inputs, trace=False)
    return out


# revision 26
# speedup vs baseline: 1.1516x; 1.0243x over previous
"""Causal self-attention (RoPE) Trainium2 kernel.

Sharding: 2 batches x 16 heads = 32 (b,h) units over 8 cores -> each core
handles 1 batch x 4 heads. Column-parallel QKV + row-parallel output
projection; host sums the 4 partial outputs per batch.

All matmul operands are fp16 (1 cycle/row on the PE); fp32 PSUM accumulation.

Design notes (v2, PE-continuity focused):
  - Attention runs on 128-wide q chunks (qtc 0..15), nkc = qtc+1 key chunks
    of 128 -> only 53% of the full score matrix is computed (vs 62.5% with
    512-wide q tiles).
  - The causal mask for the single diagonal [128,128] block is applied ON
    THE PE as an extra accumulating matmul (iden^T @ masktri) into the S
    PSUM, so the S -> exp chain never touches the Vector engine.
  - exp is batched: up to 4 S subtiles share one [128,512] PSUM bank and
    one scalar-engine activation.
  - RoPE: two PSUM-reading multiplies + the pair-swap shuffle on Vector;
    the final add runs on GpSimd (idle otherwise).
  - Softmax denominator: ones-column matmuls accumulate [1,128] per q chunk;
    reciprocal on Vector, partition_broadcast on GpSimd, normalize multiply
    on Vector.
  - Emission interleaves att(h) with qk(h+1) and att(3) with the output
    projection so the PE always has dense matmul work queued behind
    exp-gated work (PE p-state drops to 1.2 GHz for 3 us after any stall).
"""

import sys

if "/opt/trn_rl_repo" not in sys.path:
    sys.path.insert(0, "/opt/trn_rl_repo")

import numpy as np

import concourse.bass as bass
import concourse.tile as tile
from concourse import bacc, mybir
from concourse.bass_utils import run_bass_kernel_spmd

F32 = mybir.dt.float32
F16 = mybir.dt.float16

B, T, C = 2, 2048, 2048
NH, HD = 16, 128
NHL = 4            # heads per core
D_LOC = NHL * HD   # 512 local head dims
N_CORES = 8
SCALE = 1.0 / float(np.sqrt(HD))
NEG = -30000.0     # big enough: exp((S+NEG)*SCALE) == 0 for |S| < ~1000

CC = C // 128      # 16 contraction chunks
KC = T // 128      # 16 key chunks
NQC = T // 128     # 16 q chunks (128 wide)

_compiled = None


def _build():
    nc = bacc.Bacc("TRN2", target_bir_lowering=False, debug=False)

    xT_d = nc.dram_tensor("xT", [C, T], F16, kind="ExternalInput")
    wq_d = nc.dram_tensor("wq", [NHL, 128, CC, 128], F16, kind="ExternalInput")
    wk_d = nc.dram_tensor("wk", [NHL, 128, CC, 128], F16, kind="ExternalInput")
    wv_d = nc.dram_tensor("wv", [128, CC, D_LOC], F16, kind="ExternalInput")
    w2_d = nc.dram_tensor("w2", [128, NHL, C], F16, kind="ExternalInput")
    cos2_d = nc.dram_tensor("cos2", [128, T], F16, kind="ExternalInput")
    sin2s_d = nc.dram_tensor("sin2s", [128, T], F16, kind="ExternalInput")
    masktri_d = nc.dram_tensor("masktri", [128, 128], F16, kind="ExternalInput")
    iden_d = nc.dram_tensor("iden", [128, 128], F16, kind="ExternalInput")
    out_d = nc.dram_tensor("out", [T, C], F16, kind="ExternalOutput")

    swap_mask = list(range(16, 32)) + list(range(16))

    with tile.TileContext(nc) as tc, \
         tc.tile_pool(name="persist", bufs=1) as persist, \
         tc.tile_pool(name="pw", bufs=4) as pw, \
         tc.tile_pool(name="rope", bufs=2) as prope, \
         tc.tile_pool(name="att", bufs=6) as patt, \
         tc.tile_pool(name="nrm", bufs=4) as pnrm, \
         tc.tile_pool(name="outp", bufs=4) as pout, \
         tc.tile_pool(name="pbig", bufs=5, space="PSUM") as pbig, \
         tc.tile_pool(name="psy", bufs=2, space="PSUM") as psy, \
         tc.tile_pool(name="psd", bufs=1, space="PSUM") as psd:
        # persistent tiles
        xs = persist.tile([128, CC, T], F16, tag="xs")
        # qkT split per (jc, tt) window and yT per (h, qc) chunk so every
        # producer/consumer dependency is exact (no reliance on subtile
        # tracking when emission interleaves phases).
        qkTt = {}
        for jc in range(8):
            for tt in range(4):
                qkTt[(jc, tt)] = persist.tile(
                    [128, 512], F16, tag=f"qkT{jc}_{tt}",
                    name=f"qkT{jc}_{tt}")
        yTt = {}
        for h in range(NHL):
            for qc in range(NQC):
                yTt[(h, qc)] = persist.tile(
                    [128, 128], F16, tag=f"yT{h}_{qc}", name=f"yT{h}_{qc}")
        v_sb = persist.tile([128, KC, D_LOC], F16, tag="vsb")

        def qk_ap(jc, t0, t1):
            # [t0, t1) must lie within one 512 window
            tt = t0 // 512
            return qkTt[(jc, tt)][:, t0 - tt * 512:t1 - tt * 512]
        masktri_sb = persist.tile([128, 128], F16, tag="masktri")
        iden_sb = persist.tile([128, 128], F16, tag="iden")
        cos2 = persist.tile([128, T], F16, tag="cos2")
        sin2s = persist.tile([128, T], F16, tag="sin2s")
        w2_sb = persist.tile([128, NHL, C], F16, tag="w2")
        wv_sb = persist.tile([128, CC, D_LOC], F16, tag="wv")
        allones_sb = persist.tile([128, 128], F16, tag="allones")
        nc.vector.memset(allones_sb, 1.0)

        def emit_xs_dma(tq, eng=None):
            eng = eng or nc.sync
            t0, t1 = tq * 512, (tq + 1) * 512
            for cc in range(CC):
                eng.dma_start(
                    out=xs[:, cc, t0:t1],
                    in_=xT_d.ap()[cc * 128:(cc + 1) * 128, t0:t1],
                )

        def emit_trig_dma(eng=None):
            eng = eng or nc.sync
            for half in range(2):
                t0, t1 = half * (T // 2), (half + 1) * (T // 2)
                eng.dma_start(out=cos2[:, t0:t1],
                              in_=cos2_d.ap()[:, t0:t1])
                eng.dma_start(out=sin2s[:, t0:t1],
                              in_=sin2s_d.ap()[:, t0:t1])

        def emit_wv_dma(eng=None):
            eng = eng or nc.sync
            for g in range(8):  # wv in 8 chunks of 2 cc
                eng.dma_start(out=wv_sb[:, 2 * g:2 * (g + 1), :],
                              in_=wv_d.ap()[:, 2 * g:2 * (g + 1), :])

        def emit_misc_dma():
            nc.sync.dma_start(out=masktri_sb, in_=masktri_d.ap())
            nc.sync.dma_start(out=iden_sb, in_=iden_d.ap())
            for hh in range(NHL):  # w2 in 4 chunks
                nc.sync.dma_start(out=w2_sb[:, hh, :],
                                  in_=w2_d.ap()[:, hh, :])

        # ---------------- V = x @ Wv, [t, d] layout ----------------
        def emit_v_tile(tch):
            pv = pbig.tile([128, 512], F32, tag="big", name=f"pv{tch}")
            for cc in range(CC):
                nc.tensor.matmul(
                    pv,
                    xs[:, cc, tch * 128:(tch + 1) * 128],
                    wv_sb[:, cc, :],
                    start=(cc == 0), stop=(cc == CC - 1),
                )
            nc.scalar.copy(v_sb[:, tch, :], pv)

        # ---------------- QK^T tiles + RoPE ----------------
        w_tiles = {}

        def emit_qk_dma(h):
            for jc in (h, 4 + h):
                w_src = (wq_d if jc < 4 else wk_d).ap()[jc % 4]
                w_sb = pw.tile([128, CC, 128], F16, tag="w",
                               name=f"w_sb{jc}")
                for g in range(4):
                    nc.sync.dma_start(out=w_sb[:, 4 * g:4 * (g + 1), :],
                                      in_=w_src[:, 4 * g:4 * (g + 1), :])
                w_tiles[jc] = w_sb

        def qk_tile(jc, tt):
            w_sb = w_tiles[jc]
            gt0 = tt * 512
            ps = pbig.tile([128, 512], F32, tag="big", name=f"psqk{jc}_{tt}")
            for cc in range(CC):
                nc.tensor.matmul(
                    ps, w_sb[:, cc, :],
                    xs[:, cc, gt0:gt0 + 512],
                    start=(cc == 0), stop=(cc == CC - 1),
                )
            u = prope.tile([128, 512], F16, tag="u", name=f"u{jc}{tt}")
            v = prope.tile([128, 512], F16, tag="v", name=f"v{jc}{tt}")
            w = prope.tile([128, 512], F16, tag="w", name=f"w{jc}{tt}")
            nc.vector.tensor_mul(u, ps, cos2[:, gt0:gt0 + 512])
            nc.vector.tensor_mul(v, ps, sin2s[:, gt0:gt0 + 512])
            nc.vector.stream_shuffle(w, v, swap_mask)
            nc.vector.tensor_add(qkTt[(jc, tt)], u, w)

        def qk_emitters(h):
            return [(lambda jc=jc, tt=tt: qk_tile(jc, tt))
                    for jc in (h, 4 + h) for tt in range(4)]

        # ---------------- attention, 128-wide q chunks ----------------
        def att_units(h):
            """Returns list of (emit_S, emit_YD) closure pairs."""
            units = []
            state = {}

            def make_S(qtc, gi, group):
                def emit_S():
                    sps4 = pbig.tile([128, 512], F32, tag="big",
                                     name=f"sps{h}_{qtc}_{gi}")
                    a4 = patt.tile([128, 512], F16, tag="a",
                                   name=f"a{h}_{qtc}_{gi}")
                    for j, kc in enumerate(group):
                        nc.tensor.matmul(
                            sps4[:, j * 128:(j + 1) * 128],
                            qk_ap(4 + h, kc * 128, (kc + 1) * 128),
                            qk_ap(h, qtc * 128, (qtc + 1) * 128),
                            start=True, stop=(kc != qtc),
                        )
                        if kc == qtc:
                            nc.tensor.matmul(
                                sps4[:, j * 128:(j + 1) * 128],
                                iden_sb, masktri_sb,
                                start=False, stop=True,
                            )
                    wdt = len(group) * 128
                    nc.scalar.activation(
                        a4[:, :wdt], sps4[:, :wdt],
                        mybir.ActivationFunctionType.Exp, scale=SCALE,
                    )
                    state[(qtc, gi)] = a4
                return emit_S

            def make_YD(qtc, gi, group, ngroups):
                def emit_YD():
                    if gi == 0:
                        state[("y", qtc)] = psy.tile(
                            [128, 128], F32, tag="y", name=f"yps{h}_{qtc}")
                        state[("d", qtc)] = psd.tile(
                            [128, 128], F32, tag="d", name=f"dps{h}_{qtc}")
                    yps = state[("y", qtc)]
                    dps = state[("d", qtc)]
                    a4 = state.pop((qtc, gi))
                    for j, kc in enumerate(group):
                        nc.tensor.matmul(
                            yps, v_sb[:, kc, h * HD:(h + 1) * HD],
                            a4[:, j * 128:(j + 1) * 128],
                            start=(kc == 0), stop=(kc == qtc),
                        )
                    # all-ones lhsT: every output row = the column sums, so
                    # dps is the denominator already broadcast to 128 rows
                    for j, kc in enumerate(group):
                        nc.tensor.matmul(
                            dps, allones_sb,
                            a4[:, j * 128:(j + 1) * 128],
                            start=(kc == 0), stop=(kc == qtc),
                        )
                    if gi == ngroups - 1:
                        rb = pnrm.tile([128, 128], F32, tag="rb",
                                       name=f"rb{h}{qtc}")
                        nc.vector.reciprocal_approx_fast(rb, dps)
                        nc.vector.tensor_mul(yTt[(h, qtc)], yps, rb)
                return emit_YD

            for qtc in range(NQC):
                kcs = list(range(qtc + 1))
                groups = [kcs[i:i + 4] for i in range(0, len(kcs), 4)]
                for gi, group in enumerate(groups):
                    units.append((make_S(qtc, gi, group),
                                  make_YD(qtc, gi, group, len(groups))))
            return units

        # ---------------- output projection ----------------
        def proj_unit(qc):
            for ct in range(C // 512):
                ops = pbig.tile([128, 512], F32, tag="big",
                                name=f"ops{qc}{ct}")
                for hh in range(NHL):
                    nc.tensor.matmul(
                        ops,
                        yTt[(hh, qc)],
                        w2_sb[:, hh, ct * 512:(ct + 1) * 512],
                        start=(hh == 0), stop=(hh == NHL - 1),
                    )
                osb = pout.tile([128, 512], F16, tag="o",
                                name=f"osb{qc}{ct}")
                # drain half on Vector, half on Scalar: halves the latency
                # until the PSUM buf frees and splits the copy load
                nc.vector.tensor_copy(osb[:, 0:256], ops[:, 0:256])
                nc.scalar.copy(osb[:, 256:512], ops[:, 256:512])
                dst = out_d.ap()[qc * 128:(qc + 1) * 128,
                                 ct * 512:(ct + 1) * 512]
                if qc == NQC - 1:
                    # split the last transfers across queues: shorter tail
                    for qq in range(4):
                        nc.sync.dma_start(
                            out=dst[:, qq * 128:(qq + 1) * 128],
                            in_=osb[:, qq * 128:(qq + 1) * 128])
                elif qc == NQC - 2:
                    for qq in range(2):
                        nc.sync.dma_start(
                            out=dst[:, qq * 256:(qq + 1) * 256],
                            in_=osb[:, qq * 256:(qq + 1) * 256])
                else:
                    nc.sync.dma_start(out=dst, in_=osb)

        # ---------------- software-pipelined emission ----------------
        LOOK = 3

        def run_phase(units, extras, period):
            """Pipeline att units (S ahead of YD by LOOK); sprinkle dense
            `extras` closures every `period` units to cover exp latency."""
            n = len(units)
            ei = 0
            step = 0
            for i in range(n + LOOK):
                if i < n:
                    units[i][0]()
                if i >= LOOK:
                    units[i - LOOK][1]()
                step += 1
                if extras and step % period == 0 and ei < len(extras):
                    extras[ei]()
                    ei += 1
            while extras and ei < len(extras):
                extras[ei]()
                ei += 1

        def run_tail(units, ends, look=2):
            # shallower lookahead: leaves a PSUM buf free so proj tiles
            # double-buffer against their drain latency
            n = len(units)
            done_proj = 0
            for i in range(n + look):
                if i < n:
                    units[i][0]()
                if i >= look:
                    units[i - look][1]()
                while done_proj < NQC and i - look + 1 >= ends[done_proj]:
                    proj_unit(done_proj)
                    done_proj += 1
            while done_proj < NQC:
                proj_unit(done_proj)
                done_proj += 1

        # --- prologue: spread DMA triggers over SP + Scalar sequencers
        # (565-667ns per trigger serializes; SP alone would gate the start),
        # ordered by first PE use; V tiles interleave into qk0 so each xs
        # quarter's deadline moves later ---
        for jc in (0, 4):
            w_tiles[jc] = pw.tile([128, CC, 128], F16, tag="w",
                                  name=f"w_sb{jc}")
        # SP: first trig window, head-0 Q weights + xs q0 interleaved
        nc.sync.dma_start(out=cos2[:, 0:512], in_=cos2_d.ap()[:, 0:512])
        nc.sync.dma_start(out=sin2s[:, 0:512], in_=sin2s_d.ap()[:, 0:512])
        wq0 = wq_d.ap()[0]
        for g in range(4):
            nc.sync.dma_start(out=w_tiles[0][:, 4 * g:4 * (g + 1), :],
                              in_=wq0[:, 4 * g:4 * (g + 1), :])
            for cc in range(4 * g, 4 * g + 4):
                nc.sync.dma_start(
                    out=xs[:, cc, 0:512],
                    in_=xT_d.ap()[cc * 128:(cc + 1) * 128, 0:512])
        # Scalar: wv (needed by interleaved V tiles), head-0 K weights,
        # xs q1, remaining trig windows
        emit_wv_dma(nc.scalar)
        wk0 = wk_d.ap()[0]
        for g in range(4):
            nc.scalar.dma_start(out=w_tiles[4][:, 4 * g:4 * (g + 1), :],
                                in_=wk0[:, 4 * g:4 * (g + 1), :])
        emit_xs_dma(1, nc.scalar)
        for t0 in range(512, T, 512):
            nc.scalar.dma_start(out=cos2[:, t0:t0 + 512],
                                in_=cos2_d.ap()[:, t0:t0 + 512])
            nc.scalar.dma_start(out=sin2s[:, t0:t0 + 512],
                                in_=sin2s_d.ap()[:, t0:t0 + 512])
        # SP continues: xs q2, q3, misc
        emit_xs_dma(2, nc.sync)
        emit_xs_dma(3, nc.sync)
        emit_misc_dma()
        # qk0 with early V tiles woven in
        vt = 0
        for tt in range(4):
            qk_tile(0, tt)
            qk_tile(4, tt)
            for _ in range(2):
                if vt < 6:
                    emit_v_tile(vt)
                    vt += 1
        for tch in range(vt, KC):
            emit_v_tile(tch)
        for h in range(NHL - 1):
            emit_qk_dma(h + 1)
            run_phase(att_units(h), qk_emitters(h + 1), period=5)
        u3 = att_units(NHL - 1)
        ends = []
        acc = 0
        for q in range(NQC):
            acc += (q // 4) + 1
            ends.append(acc)
        run_tail(u3, ends)

    nc.compile()
    return nc


def _prep_core_inputs(core, x16, W_attn, W_proj, cos2, sin2s, masktri, iden):
    b = core // 4
    g = core % 4
    heads = [g * NHL + i for i in range(NHL)]
    # stream_shuffle permutes within 32-partition blocks only: lay out each
    # block as [re pairs 16b..16b+15 | im pairs 16b..16b+15]
    perm = np.concatenate(
        [np.r_[2 * (16 * blk + np.arange(16)),
               2 * (16 * blk + np.arange(16)) + 1]
         for blk in range(4)]
    )

    xT = np.ascontiguousarray(x16[b].T)

    def qk_blocks(base):
        blocks = []
        for h in heads:
            blk = W_attn[:, base + h * HD: base + (h + 1) * HD][:, perm]
            blocks.append(blk.reshape(CC, 128, HD).transpose(1, 0, 2))
        return np.ascontiguousarray(np.stack(blocks, axis=0)).astype(np.float16)

    wq = qk_blocks(0)
    wk = qk_blocks(C)
    wv = np.concatenate(
        [W_attn[:, 2 * C + h * HD: 2 * C + (h + 1) * HD] for h in heads],
        axis=1,
    )  # (C, D_LOC)
    wv = np.ascontiguousarray(
        wv.reshape(CC, 128, D_LOC).transpose(1, 0, 2)).astype(np.float16)
    w2 = np.ascontiguousarray(
        np.stack([W_proj[h * HD:(h + 1) * HD, :] for h in heads], axis=0)
        .transpose(1, 0, 2)
    ).astype(np.float16)
    return {
        "xT": xT, "wq": wq, "wk": wk, "wv": wv, "w2": w2,
        "cos2": cos2, "sin2s": sin2s, "masktri": masktri, "iden": iden,
    }


def _run(inputs, trace=False):
    global _compiled
    x = np.asarray(inputs["x"], dtype=np.float32)
    W_attn = np.asarray(inputs["W_attn"], dtype=np.float32)
    W_proj = np.asarray(inputs["W_proj"], dtype=np.float32)
    fc = np.asarray(inputs["freqs_cos"], dtype=np.float32)
    fs = np.asarray(inputs["freqs_sin"], dtype=np.float32)

    x16 = x.astype(np.float16)

    cosT = np.ascontiguousarray(fc.T)            # (64, T)
    sinT = np.ascontiguousarray(fs.T)
    # per 32-partition block b: partitions [0:16] carry cos/sin of pairs
    # 16b..16b+15 (re half, +sin), [16:32] the same freqs (im half, -sin)
    cos2 = np.concatenate(
        [np.concatenate([cosT[16 * blk:16 * (blk + 1)]] * 2, axis=0)
         for blk in range(4)], axis=0)           # (128, T)
    sin2s = np.concatenate(
        [np.concatenate([sinT[16 * blk:16 * (blk + 1)],
                         -sinT[16 * blk:16 * (blk + 1)]], axis=0)
         for blk in range(4)], axis=0)
    cos2 = np.ascontiguousarray(cos2).astype(np.float16)
    sin2s = np.ascontiguousarray(sin2s).astype(np.float16)

    ki = np.arange(128)[:, None]
    qi = np.arange(128)[None, :]
    masktri = np.ascontiguousarray(
        np.where(ki <= qi, 0.0, NEG).astype(np.float16))  # (128, 128)
    iden = np.ascontiguousarray(np.eye(128, dtype=np.float16))

    if _compiled is None:
        _compiled = _build()
    nc = _compiled

    in_maps = [
        _prep_core_inputs(c, x16, W_attn, W_proj, cos2, sin2s, masktri, iden)
        for c in range(N_CORES)
    ]
    res = run_bass_kernel_spmd(
        nc, in_maps, core_ids=list(range(N_CORES)), trace=trace)

    out = np.zeros((B, T, C), dtype=np.float32)
    for c in range(N_CORES):
        out[c // 4] += res.results[c]["out"]
    return out, res


def kernel(**inputs) -> np.ndarray:
    out, _ = _run(inputs, trace=False)
    return out
